# revision 9
# baseline (speedup 1.0000x reference)
"""Bidirectional-LSTM (degenerate variant) Trainium2 kernel.

Reference semantics (see harness): for the forward direction only the last
timestep matters (h/c never update), and the backward direction is an
h-only recurrence (c stays zero), so only the i/g/o gates are ever used:

    h_fwd = sig(o) * tanh(sig(i) * tanh(g)),  gates = x_last @ W_ih_f.T + b_f
    h_bwd: scan t = S-1..0 with
        gates = x_t @ W_ih_b.T + b_b + h @ W_hh_b.T   (f-gate unused)
        h     = sig(o) * tanh(sig(i) * tanh(g))
    out = [h_fwd | h_bwd]  -> [256, 4096]

Distribution: pure data-parallel over batch (32 per core, 8 cores), weights
replicated. Per core:
  phase 1: m-outer fused pipeline — embedding gather (indirect DMA, table
           pre-cast to bf16 on host) -> PE-transpose -> input projection
           xg = X @ Wi + b (bf16, Wi SBUF-resident) -> xg to DRAM; fwd cell
           at the end with Wf streamed per-g. Wr recurrence weights are
           prefetched k-tile-granular: 6 tiles trickle in on the scalar DMA
           queue during the m-loop, the rest right after the fwd cell, in
           step-1 consumption order, so phase R starts with almost no
           weight-load bubble.
  phase R: 128-step recurrence. gates = Wr.T @ h via 4 col-tiled concurrent
           M=32 matmuls (bf16), two chunk-major k-order sweeps (i,g then o).
           All three xg contributions enter the PE as start=True
           shifted-identity fold rounds issued one step AHEAD (before the
           previous step's transposes in the PE FIFO), so they execute in
           the act-ladder dependency gap; xg tiles double-buffer and load
           split across the sync+gpsimd DMA queues a full step early.
           The i/g activation chain hides under the o matmul stream; h is
           re-transposed per 128-col chunk so the next sweep starts as the
           first transposed chunk lands.

Gate columns are host-permuted into 4 groups of (i|g|o) x 512 hid dims so
each PSUM column-group j directly yields h[:, 512j:512j+512].
"""

import numpy as np
import ml_dtypes

import concourse.bass as bass
import concourse.bacc as bacc
import concourse.mybir as mybir
import concourse.tile as tile
from concourse.masks import make_identity

VOCAB, EMB, HID = 50000, 1024, 2048
BATCH, SEQ = 256, 128
NCORES = 8
BLOC = BATCH // NCORES            # 32 batch rows per core
NTOK = BLOC * SEQ                 # 4096 tokens per core
NG = 4                            # PSUM column groups
GC = 3 * HID // NG                # 1536 gate cols per group (i|g|o x 512)
HG = HID // NG                    # 512 hid dims per group
G3 = 3 * HID                      # 6144 total igo gate cols
MT = NTOK // 128                  # 32 token m-tiles
KT_E = EMB // 128                 # 8 k-tiles for input projection
KT_H = HID // 128                 # 16 k-tiles for recurrence
NWR1 = 6                          # Wr k-tiles prefetched during m-loop

F32 = mybir.dt.float32
BF16 = mybir.dt.bfloat16
I32 = mybir.dt.int32

N_STEPS = SEQ  # overridable for mini builds

# chunk-major k order: the 4 k-tiles living in hT chunk 0 run first, so a
# sweep starts at full rate as soon as the previous step's first transposed
# chunk lands. Also the Wr k-tile DMA issue order.
K_ORDER = [q + NG * r for q in range(NG) for r in range(NG)]


def build(n_steps=None):
    n_steps = n_steps or N_STEPS
    nc = bacc.Bacc("TRN2", target_bir_lowering=False, debug=False,
                   num_devices=NCORES)

    tok = nc.dram_tensor("tok", [NTOK, 1], I32, kind="ExternalInput")
    table = nc.dram_tensor("table", [VOCAB, EMB], BF16, kind="ExternalInput")
    Wi = nc.dram_tensor("Wi", [EMB, G3], BF16, kind="ExternalInput")
    Wf = nc.dram_tensor("Wf", [EMB, G3], BF16, kind="ExternalInput")
    Wr = nc.dram_tensor("Wr", [HID, G3], BF16, kind="ExternalInput")
    bias_b = nc.dram_tensor("bias_b", [128, G3], BF16, kind="ExternalInput")
    bias_f = nc.dram_tensor("bias_f", [128, G3], BF16, kind="ExternalInput")
    out = nc.dram_tensor("out", [BLOC, 2 * HID], F32, kind="ExternalOutput")

    xgd = nc.dram_tensor("xgd", [NTOK, G3], BF16)         # internal

    wr_tiles = {}

    with tile.TileContext(nc) as tc:
        with tc.tile_pool(name="wr1", bufs=1) as wr1p:
            # ------- phase 1: gather + transpose + input projection -------
            with tc.tile_pool(name="p1s", bufs=1) as p1s, \
                 tc.tile_pool(name="p1_ps", bufs=2, space="PSUM") as p1_ps:
                with tc.tile_pool(name="p1w", bufs=1) as p1w, \
                     tc.tile_pool(name="p1", bufs=2) as p1, \
                     tc.tile_pool(name="p1x", bufs=2) as p1x, \
                     tc.tile_pool(name="p1t_ps", bufs=2, space="PSUM") as p1t_ps:
                    ident = p1s.tile([128, 128], BF16)
                    make_identity(nc, ident[:])
                    # first two gathers go out before the weight loads; Wi is
                    # split across the sync+scalar DMA queues so the m-loop
                    # can start ~35us in.
                    pre_idx, pre_x = [], []
                    for m in range(2):
                        idx_sb = p1.tile([128, 1], I32, tag="idx")
                        nc.sync.dma_start(out=idx_sb[:], in_=tok[m * 128:(m + 1) * 128, :])
                        x_sb = p1x.tile([128, EMB], BF16, tag="x")
                        nc.gpsimd.indirect_dma_start(
                            out=x_sb[:], out_offset=None, in_=table[:, :],
                            in_offset=bass.IndirectOffsetOnAxis(ap=idx_sb[:, :1], axis=0))
                        pre_idx.append(idx_sb)
                        pre_x.append(x_sb)
                    wi_sb = []
                    for g in range(NG):
                        w_g = p1w.tile([128, KT_E, GC], BF16, tag=f"wi{g}")
                        q = nc.sync if g in (0, 2) else nc.scalar
                        q.dma_start(
                            out=w_g[:],
                            in_=Wi[:, GC * g:GC * (g + 1)].rearrange("(k p) c -> p k c", p=128))
                        wi_sb.append(w_g)
                    bia_sb = p1s.tile([128, G3], BF16, tag="bia")
                    nc.scalar.dma_start(out=bia_sb[:], in_=bias_b[:, :])
                    # Wr k-tile prefetch: 6 tiles trickle in on the scalar
                    # queue (behind Wi g1/g3+bias) while the m-loop runs.
                    for k in K_ORDER[:NWR1]:
                        w_k = wr1p.tile([128, G3], BF16, tag=f"wr{k}")
                        nc.scalar.dma_start(out=w_k[:], in_=Wr[128 * k:128 * (k + 1), :])
                        wr_tiles[k] = w_k
                    xt0_sb = p1s.tile([128, EMB], BF16)

                    for m in range(MT):
                        if m < 2:
                            x_sb = pre_x[m]
                        else:
                            # idx+gather both on the gpsimd queue (FIFO keeps
                            # them ordered) so the sync queue stays free for
                            # xgd writes.
                            idx_sb = p1.tile([128, 1], I32, tag="idx")
                            nc.gpsimd.dma_start(out=idx_sb[:], in_=tok[m * 128:(m + 1) * 128, :])
                            x_sb = p1x.tile([128, EMB], BF16, tag="x")
                            nc.gpsimd.indirect_dma_start(
                                out=x_sb[:], out_offset=None, in_=table[:, :],
                                in_offset=bass.IndirectOffsetOnAxis(ap=idx_sb[:, :1], axis=0))
                        xt_sb = p1x.tile([128, EMB], BF16, tag="xt")
                        for q in range(KT_E):
                            t_ps = p1t_ps.tile([128, 128], BF16, space="PSUM", tag="tps")
                            nc.tensor.transpose(out=t_ps[:], in_=x_sb[:, 128 * q:128 * (q + 1)],
                                                identity=ident[:])
                            nc.vector.tensor_copy(xt_sb[:, 128 * q:128 * (q + 1)], t_ps[:])
                        if m == 0:
                            nc.vector.tensor_copy(xt0_sb[:], xt_sb[:])
                        for g in range(NG):
                            ps = p1_ps.tile([128, GC], F32, space="PSUM", tag="ps")
                            for k in range(KT_E):
                                for c in range(3):
                                    nc.tensor.matmul(
                                        ps[:, 512 * c:512 * (c + 1)],
                                        lhsT=xt_sb[:, 128 * k:128 * (k + 1)],
                                        rhs=wi_sb[g][:, k, 512 * c:512 * (c + 1)],
                                        start=(k == 0), stop=(k == KT_E - 1))
                            xg_sb = p1.tile([128, GC], BF16, tag="xg")
                            nc.vector.tensor_add(xg_sb[:], ps[:],
                                                 bia_sb[:, GC * g:GC * (g + 1)])
                            nc.sync.dma_start(
                                out=xgd[m * 128:(m + 1) * 128, GC * g:GC * (g + 1)],
                                in_=xg_sb[:])

                # ---- forward cell (tokens 0..32 = original last step) ----
                # Wf streamed in per-g chunks, double-buffered against the
                # MMs; its pools open only after the m-loop pools close.
                with tc.tile_pool(name="p1wf", bufs=1) as p1wf, \
                     tc.tile_pool(name="p1f", bufs=1) as p1f:
                    bif_sb = p1f.tile([BLOC, G3], BF16, tag="bif")
                    nc.gpsimd.dma_start(out=bif_sb[:], in_=bias_f[0:BLOC, :])
                    for g in range(NG):
                        wf_sb = p1wf.tile([128, KT_E, GC], BF16, tag="wf", bufs=2)
                        wf_src = Wf[:, GC * g:GC * (g + 1)].rearrange("(k p) c -> p k c", p=128)
                        qt = KT_E // 4
                        for h4 in range(4):
                            q = nc.scalar if h4 % 2 == 0 else nc.sync
                            q.dma_start(out=wf_sb[:, h4 * qt:(h4 + 1) * qt, :],
                                        in_=wf_src[:, h4 * qt:(h4 + 1) * qt, :])
                        psf = p1_ps.tile([128, GC], F32, space="PSUM", tag="ps")
                        for k in range(KT_E):
                            for c in range(3):
                                nc.tensor.matmul(
                                    psf[0:BLOC, 512 * c:512 * (c + 1)],
                                    lhsT=xt0_sb[:, 128 * k:128 * k + BLOC],
                                    rhs=wf_sb[:, k, 512 * c:512 * (c + 1)],
                                    start=(k == 0), stop=(k == KT_E - 1))
                        gf = p1f.tile([BLOC, GC], F32, tag="gf")
                        nc.vector.tensor_add(gf[:], psf[0:BLOC, :],
                                             bif_sb[:, GC * g:GC * (g + 1)])
                        af = p1f.tile([BLOC, HG], F32, tag="af")
                        bf = p1f.tile([BLOC, HG], F32, tag="bff")
                        cf = p1f.tile([BLOC, HG], F32, tag="cf")
                        nc.scalar.activation(af[:], gf[:, 0:HG],
                                             mybir.ActivationFunctionType.Sigmoid)
                        nc.scalar.activation(bf[:], gf[:, HG:2 * HG],
                                             mybir.ActivationFunctionType.Tanh)
                        nc.scalar.activation(cf[:], gf[:, 2 * HG:3 * HG],
                                             mybir.ActivationFunctionType.Sigmoid)
                        nc.vector.tensor_mul(af[:], af[:], bf[:])
                        nc.scalar.activation(af[:], af[:],
                                             mybir.ActivationFunctionType.Tanh)
                        nc.vector.tensor_mul(af[:], cf[:], af[:])
                        nc.sync.dma_start(out=out[:, HG * g:HG * (g + 1)], in_=af[:])

            # remaining Wr k-tiles, split across both HW DMA queues, in the
            # order step 1 will consume them.
            with tc.tile_pool(name="wr2", bufs=1) as wr2p:
                tc.strict_bb_all_engine_barrier()
                wr2_loads = []
                for i, k in enumerate(K_ORDER[NWR1:]):
                    w_k = wr2p.tile([128, G3], BF16, tag=f"wr{k}")
                    wr2_loads.append((nc.sync if i % 2 == 0 else nc.scalar, w_k, k))
                    wr_tiles[k] = w_k
                # ---------------- phase R: recurrence ----------------
                with tc.tile_pool(name="pr", bufs=2) as pr, \
                     tc.tile_pool(name="pr1", bufs=1) as pr1, \
                     tc.tile_pool(name="prh", bufs=8) as prh, \
                     tc.tile_pool(name="pr_ps", bufs=2, space="PSUM") as pr_ps, \
                     tc.tile_pool(name="prt_ps", bufs=2, space="PSUM") as prt_ps:
                    identb = pr1.tile([128, 128], BF16)
                    make_identity(nc, identb[:])

                    a_t = pr1.tile([128, HG], F32)
                    b_t = pr1.tile([128, HG], F32)

                    def load_xg(s):
                        # split across sync + gpsimd DMA queues; steps 1-2
                        # stay off the sync queue (busy with the wr2 tiles).
                        xg_sb = pr.tile([128, GC], BF16, tag="xgs")
                        for j in range(NG):
                            q = nc.gpsimd if (s in (1, 2) or j >= 2) else nc.sync
                            q.dma_start(
                                out=xg_sb[BLOC * j:BLOC * (j + 1), :],
                                in_=xgd[BLOC * s:BLOC * (s + 1), GC * j:GC * (j + 1)])
                        return xg_sb

                    def issue_folds(xg_sb):
                        """The three xg fold rounds (start=True) for a step:
                        shifted-identity stationary selects xg rows
                        32j..32j+32 for psum region j — exact, and h-
                        independent so the PE chews them during the previous
                        step's act-ladder dependency gap."""
                        ps_i = pr_ps.tile([128, 512], F32, space="PSUM", tag="gps0")
                        ps_g = pr_ps.tile([128, 512], F32, space="PSUM", tag="gps1")
                        ps_o = pr_ps.tile([128, 512], F32, space="PSUM", tag="gps2")
                        for c, ps_c in ((0, ps_i), (1, ps_g), (2, ps_o)):
                            for j in range(NG):
                                nc.tensor.matmul(
                                    ps_c[BLOC * j:BLOC * (j + 1), :],
                                    lhsT=identb[:, BLOC * j:BLOC * (j + 1)],
                                    rhs=xg_sb[:, 512 * c:512 * (c + 1)],
                                    start=True, stop=False,
                                    tile_position=(0, BLOC * j),
                                    skip_group_check=True)
                        return ps_i, ps_g, ps_o

                    def act_part1(gi_ap, gg_ap):
                        """v = tanh(sig(i)*tanh(g)) -> a_t. Runs on ACT/DVE while
                        the o-bank matmuls stream on the PE."""
                        nc.scalar.activation(a_t[:], gi_ap,
                                             mybir.ActivationFunctionType.Sigmoid)
                        nc.scalar.activation(b_t[:], gg_ap,
                                             mybir.ActivationFunctionType.Tanh)
                        nc.vector.tensor_mul(a_t[:], a_t[:], b_t[:])      # u = sig(i)*tanh(g)
                        nc.scalar.activation(a_t[:], a_t[:],
                                             mybir.ActivationFunctionType.Tanh)  # v

                    def act_part2(go_ap, store_out=False, pe_only=False):
                        if store_out:
                            nc.scalar.activation(b_t[:], go_ap,
                                                 mybir.ActivationFunctionType.Sigmoid)
                            h_t = pr.tile([128, HG], F32, tag="hfin", bufs=1)
                            nc.vector.tensor_mul(h_t[:], b_t[:], a_t[:])
                            for j in range(NG):
                                nc.sync.dma_start(
                                    out=out[:, HID + HG * j:HID + HG * (j + 1)],
                                    in_=h_t[BLOC * j:BLOC * (j + 1), :])
                            return None
                        # chunk-pipelined: per-128-col sigmoid -> mul -> PE
                        # transpose -> copy, so the first transpose starts
                        # ~0.35us earlier than a full-width sigmoid allows
                        # chunk 0 transposes on the PE (shortest latency to
                        # unblock the next sweep); chunks 1-3 on the DMA xbar
                        # whose latency hides behind the chunk-0 matmul groups.
                        # The xbar issues go AFTER the whole sig/mul ladder in
                        # program order: a DMA's semaphore wait blocks the
                        # issuing engine's queue, and it must not hold up the
                        # later sigmoids.
                        hTs, deferred = [], []
                        for q in range(NG):
                            sl = slice(128 * q, 128 * (q + 1))
                            b_q = pr.tile([128, 128], BF16, tag="bq", bufs=2)
                            nc.scalar.activation(b_q[:], go_ap[:, sl],
                                                 mybir.ActivationFunctionType.Sigmoid)
                            h_q = pr.tile([128, 128], BF16, tag="h", bufs=3)
                            nc.vector.tensor_mul(h_q[:], b_q[:], a_t[:, sl])
                            hT_q = prh.tile([128, 128], BF16, tag="hT")
                            if q == 0 or pe_only:
                                t_ps = prt_ps.tile([128, 128], BF16, space="PSUM", tag="tps")
                                nc.tensor.transpose(out=t_ps[:], in_=h_q[:],
                                                    identity=identb[:])
                                nc.vector.tensor_copy(hT_q[:], t_ps[:])
                            else:
                                deferred.append((q, h_q, hT_q))
                            hTs.append(hT_q)
                        for q, h_q, hT_q in deferred:
                            eng = nc.sync if q == 2 else nc.scalar
                            eng.dma_start_transpose(out=hT_q[:], in_=h_q[:])
                        return hTs

                    # step 0: h=0 -> gates are just xg. Issued BEFORE the
                    # wr2 weight DMAs so its ladder isn't queued behind their
                    # semaphore waits on the hw DMA queues.
                    xg0 = load_xg(0)
                    act_part1(xg0[:, 0:HG], xg0[:, HG:2 * HG])
                    hT = act_part2(xg0[:, 2 * HG:3 * HG], pe_only=True)
                    # step 1 folds go into the PE queue right behind step 0's
                    # transposes
                    xg_nxt = load_xg(1)
                    folds = issue_folds(xg_nxt)
                    for q, w_k, k in wr2_loads:
                        q.dma_start(out=w_k[:], in_=Wr[128 * k:128 * (k + 1), :])

                    for s in range(1, n_steps):
                        ps_i, ps_g, ps_o = folds
                        # sweep 1: i and g banks, k-outer so each hT stationary
                        # load serves 8 matmuls; o bank in a second sweep so the
                        # i/g activation chain overlaps the o matmul stream.
                        for ki, k in enumerate(K_ORDER):
                            lhs = hT[k % NG][:, BLOC * (k // NG):BLOC * (k // NG) + BLOC]
                            for c, ps_c in ((0, ps_i), (1, ps_g)):
                                for j in range(NG):
                                    nc.tensor.matmul(
                                        ps_c[BLOC * j:BLOC * (j + 1), :],
                                        lhsT=lhs,
                                        rhs=wr_tiles[k][:, GC * j + 512 * c:GC * j + 512 * (c + 1)],
                                        start=False, stop=(ki == KT_H - 1),
                                        tile_position=(0, BLOC * j),
                                        skip_group_check=True)
                        # sweep 2: o bank (xg contribution already folded in
                        # as the start=True round).
                        for ki, k in enumerate(K_ORDER):
                            lhs = hT[k % NG][:, BLOC * (k // NG):BLOC * (k // NG) + BLOC]
                            for j in range(NG):
                                nc.tensor.matmul(
                                    ps_o[BLOC * j:BLOC * (j + 1), :],
                                    lhsT=lhs,
                                    rhs=wr_tiles[k][:, GC * j + 1024:GC * j + 1536],
                                    start=False, stop=(ki == KT_H - 1),
                                    tile_position=(0, BLOC * j),
                                    skip_group_check=True)
                        # next step's xg load + folds enter the PE queue HERE —
                        # before this step's transposes — so they fill the
                        # act-ladder dependency gap (PE queue is strict FIFO).
                        if s + 1 < n_steps:
                            xg_nxt = load_xg(s + 1)
                            folds = issue_folds(xg_nxt)
                        act_part1(ps_i[:], ps_g[:])
                        hT = act_part2(ps_o[:], store_out=(s == n_steps - 1),
                                       pe_only=(s <= 2))
    nc.compile()
    return nc


_BUILT = {}


def _get_built(n_steps=None):
    key = n_steps or N_STEPS
    if key not in _BUILT:
        _BUILT[key] = build(key)
    return _BUILT[key]


def _perm():
    """Row permutation taking PyTorch (i|f|g|o)*2048 rows to 4 groups of
    (i|g|o)*512."""
    p = []
    for j in range(NG):
        for base in (0, 2 * HID, 3 * HID):  # i, g, o blocks
            p.extend(range(base + HG * j, base + HG * (j + 1)))
    return np.array(p)


def prep_inputs(inputs, embed_table, W_ih_f, W_hh_f, b_ih_f, b_hh_f,
                W_ih_b, W_hh_b, b_ih_b, b_hh_b):
    perm = _perm()
    idx = np.asarray(inputs)
    idx = np.where(idx > VOCAB, 0, idx).astype(np.int64)
    idx = np.clip(idx, 0, VOCAB - 1).astype(np.int32)

    Wi_p = np.ascontiguousarray(
        np.asarray(W_ih_b)[perm].T.astype(ml_dtypes.bfloat16))
    Wf_p = np.ascontiguousarray(
        np.asarray(W_ih_f)[perm].T.astype(ml_dtypes.bfloat16))
    Wr_p = np.ascontiguousarray(
        np.asarray(W_hh_b)[perm].T.astype(ml_dtypes.bfloat16))
    bb = (np.asarray(b_ih_b) + np.asarray(b_hh_b))[perm].astype(np.float32)
    bf = (np.asarray(b_ih_f) + np.asarray(b_hh_f))[perm].astype(np.float32)
    bias_b_t = np.ascontiguousarray(
        np.broadcast_to(bb, (128, G3)).astype(ml_dtypes.bfloat16))
    bias_f_t = np.ascontiguousarray(
        np.broadcast_to(bf, (128, G3)).astype(ml_dtypes.bfloat16))
    table = np.ascontiguousarray(
        np.asarray(embed_table).astype(ml_dtypes.bfloat16))

    in_maps = []
    for c in range(NCORES):
        sl = idx[BLOC * c:BLOC * (c + 1)]          # [32, 128]
        tok = np.ascontiguousarray(sl[:, ::-1].T.reshape(NTOK, 1))  # t-major rev
        in_maps.append({
            "tok": tok, "table": table, "Wi": Wi_p, "Wf": Wf_p, "Wr": Wr_p,
            "bias_b": bias_b_t, "bias_f": bias_f_t,
        })
    return in_maps


def kernel(**inputs) -> np.ndarray:
    from concourse.bass_utils import run_bass_kernel_spmd
    nc = _get_built()
    in_maps = prep_inputs(**inputs)
    res = run_bass_kernel_spmd(nc, in_maps, core_ids=list(range(NCORES)))
    return np.concatenate([res.results[c]["out"] for c in range(NCORES)], axis=0)


# revision 10
# speedup vs baseline: 1.0529x; 1.0529x over previous
"""Bidirectional-LSTM (degenerate variant) Trainium2 kernel.

Reference semantics (see harness): for the forward direction only the last
timestep matters (h/c never update), and the backward direction is an
h-only recurrence (c stays zero), so only the i/g/o gates are ever used:

    h_fwd = sig(o) * tanh(sig(i) * tanh(g)),  gates = x_last @ W_ih_f.T + b_f
    h_bwd: scan t = S-1..0 with
        gates = x_t @ W_ih_b.T + b_b + h @ W_hh_b.T   (f-gate unused)
        h     = sig(o) * tanh(sig(i) * tanh(g))
    out = [h_fwd | h_bwd]  -> [256, 4096]

Distribution: pure data-parallel over batch (32 per core, 8 cores), weights
replicated. Per core:
  phase 1: m-outer fused pipeline — embedding gather (indirect DMA, table
           pre-cast to bf16 on host) -> PE-transpose -> input projection
           xg = X @ Wi + b (bf16, Wi SBUF-resident) -> xg to DRAM; fwd cell
           at the end with Wf streamed per-g. Wr recurrence weights are
           prefetched k-tile-granular: 6 tiles trickle in on the scalar DMA
           queue during the m-loop, the rest right after the fwd cell, in
           step-1 consumption order, so phase R starts with almost no
           weight-load bubble.
  phase R: 128-step recurrence. gates = Wr.T @ h via 4 col-tiled concurrent
           M=32 matmuls (bf16), two chunk-major k-order sweeps (i,g then o).
           All three xg contributions enter the PE as start=True
           shifted-identity fold rounds issued one step AHEAD (before the
           previous step's transposes in the PE FIFO), so they execute in
           the act-ladder dependency gap; xg tiles double-buffer and load
           split across the sync+gpsimd DMA queues a full step early.
           The i/g activation chain hides under the o matmul stream; h is
           re-transposed per 128-col chunk so the next sweep starts as the
           first transposed chunk lands.

Gate columns are host-permuted into 4 groups of (i|g|o) x 512 hid dims so
each PSUM column-group j directly yields h[:, 512j:512j+512].
"""

import numpy as np
import ml_dtypes

import concourse.bass as bass
import concourse.bacc as bacc
import concourse.mybir as mybir
import concourse.tile as tile
from concourse.masks import make_identity

VOCAB, EMB, HID = 50000, 1024, 2048
BATCH, SEQ = 256, 128
NCORES = 8
BLOC = BATCH // NCORES            # 32 batch rows per core
NTOK = BLOC * SEQ                 # 4096 tokens per core
NG = 4                            # PSUM column groups
GC = 3 * HID // NG                # 1536 gate cols per group (i|g|o x 512)
HG = HID // NG                    # 512 hid dims per group
G3 = 3 * HID                      # 6144 total igo gate cols
MT = NTOK // 128                  # 32 token m-tiles
KT_E = EMB // 128                 # 8 k-tiles for input projection
KT_H = HID // 128                 # 16 k-tiles for recurrence
NWR1 = 6                          # Wr k-tiles prefetched during m-loop

F32 = mybir.dt.float32
BF16 = mybir.dt.bfloat16
I32 = mybir.dt.int32

N_STEPS = SEQ  # overridable for mini builds

# chunk-major k order: the 4 k-tiles living in hT chunk 0 run first, so a
# sweep starts at full rate as soon as the previous step's first transposed
# chunk lands. Also the Wr k-tile DMA issue order.
K_ORDER = [q + NG * r for q in range(NG) for r in range(NG)]


def build(n_steps=None):
    n_steps = n_steps or N_STEPS
    nc = bacc.Bacc("TRN2", target_bir_lowering=False, debug=False,
                   num_devices=NCORES)

    tok = nc.dram_tensor("tok", [NTOK, 1], I32, kind="ExternalInput")
    table = nc.dram_tensor("table", [VOCAB, EMB], BF16, kind="ExternalInput")
    Wi = nc.dram_tensor("Wi", [EMB, G3], BF16, kind="ExternalInput")
    Wf = nc.dram_tensor("Wf", [EMB, G3], BF16, kind="ExternalInput")
    Wr = nc.dram_tensor("Wr", [HID, G3], BF16, kind="ExternalInput")
    bias_b = nc.dram_tensor("bias_b", [128, G3], BF16, kind="ExternalInput")
    bias_f = nc.dram_tensor("bias_f", [128, G3], BF16, kind="ExternalInput")
    out = nc.dram_tensor("out", [BLOC, 2 * HID], F32, kind="ExternalOutput")

    xgd = nc.dram_tensor("xgd", [NTOK, G3], BF16)         # internal

    wr_tiles = {}

    with tile.TileContext(nc) as tc:
        with tc.tile_pool(name="wr1", bufs=1) as wr1p:
            # ------- phase 1: gather + transpose + input projection -------
            with tc.tile_pool(name="p1s", bufs=1) as p1s, \
                 tc.tile_pool(name="p1_ps", bufs=2, space="PSUM") as p1_ps:
                with tc.tile_pool(name="p1w", bufs=1) as p1w, \
                     tc.tile_pool(name="p1", bufs=2) as p1, \
                     tc.tile_pool(name="p1x", bufs=2) as p1x, \
                     tc.tile_pool(name="p1t_ps", bufs=2, space="PSUM") as p1t_ps:
                    ident = p1s.tile([128, 128], BF16)
                    make_identity(nc, ident[:])
                    # first two gathers go out before the weight loads; Wi is
                    # split across the sync+scalar DMA queues so the m-loop
                    # can start ~35us in.
                    pre_idx, pre_x = [], []
                    for m in range(2):
                        idx_sb = p1.tile([128, 1], I32, tag="idx")
                        nc.sync.dma_start(out=idx_sb[:], in_=tok[m * 128:(m + 1) * 128, :])
                        x_sb = p1x.tile([128, EMB], BF16, tag="x")
                        nc.gpsimd.indirect_dma_start(
                            out=x_sb[:], out_offset=None, in_=table[:, :],
                            in_offset=bass.IndirectOffsetOnAxis(ap=idx_sb[:, :1], axis=0))
                        pre_idx.append(idx_sb)
                        pre_x.append(x_sb)
                    wi_sb = []
                    for g in range(NG):
                        w_g = p1w.tile([128, KT_E, GC], BF16, tag=f"wi{g}")
                        q = nc.sync if g in (0, 2) else nc.scalar
                        q.dma_start(
                            out=w_g[:],
                            in_=Wi[:, GC * g:GC * (g + 1)].rearrange("(k p) c -> p k c", p=128))
                        wi_sb.append(w_g)
                    bia_sb = p1s.tile([128, G3], BF16, tag="bia")
                    nc.scalar.dma_start(out=bia_sb[:], in_=bias_b[:, :])
                    # Wr k-tile prefetch: 6 tiles trickle in on the scalar
                    # queue (behind Wi g1/g3+bias) while the m-loop runs.
                    for k in K_ORDER[:NWR1]:
                        w_k = wr1p.tile([128, G3], BF16, tag=f"wr{k}")
                        nc.scalar.dma_start(out=w_k[:], in_=Wr[128 * k:128 * (k + 1), :])
                        wr_tiles[k] = w_k
                    xt0_sb = p1s.tile([128, EMB], BF16)

                    for m in range(MT):
                        if m < 2:
                            x_sb = pre_x[m]
                        else:
                            # idx+gather both on the gpsimd queue (FIFO keeps
                            # them ordered) so the sync queue stays free for
                            # xgd writes.
                            idx_sb = p1.tile([128, 1], I32, tag="idx")
                            nc.gpsimd.dma_start(out=idx_sb[:], in_=tok[m * 128:(m + 1) * 128, :])
                            x_sb = p1x.tile([128, EMB], BF16, tag="x")
                            nc.gpsimd.indirect_dma_start(
                                out=x_sb[:], out_offset=None, in_=table[:, :],
                                in_offset=bass.IndirectOffsetOnAxis(ap=idx_sb[:, :1], axis=0))
                        xt_sb = p1x.tile([128, EMB], BF16, tag="xt")
                        for q in range(KT_E):
                            t_ps = p1t_ps.tile([128, 128], BF16, space="PSUM", tag="tps")
                            nc.tensor.transpose(out=t_ps[:], in_=x_sb[:, 128 * q:128 * (q + 1)],
                                                identity=ident[:])
                            nc.vector.tensor_copy(xt_sb[:, 128 * q:128 * (q + 1)], t_ps[:])
                        if m == 0:
                            nc.vector.tensor_copy(xt0_sb[:], xt_sb[:])
                        for g in range(NG):
                            ps = p1_ps.tile([128, GC], F32, space="PSUM", tag="ps")
                            for k in range(KT_E):
                                for c in range(3):
                                    nc.tensor.matmul(
                                        ps[:, 512 * c:512 * (c + 1)],
                                        lhsT=xt_sb[:, 128 * k:128 * (k + 1)],
                                        rhs=wi_sb[g][:, k, 512 * c:512 * (c + 1)],
                                        start=(k == 0), stop=(k == KT_E - 1))
                            xg_sb = p1.tile([128, GC], BF16, tag="xg")
                            nc.vector.tensor_add(xg_sb[:], ps[:],
                                                 bia_sb[:, GC * g:GC * (g + 1)])
                            nc.sync.dma_start(
                                out=xgd[m * 128:(m + 1) * 128, GC * g:GC * (g + 1)],
                                in_=xg_sb[:])

                # ---- forward cell (tokens 0..32 = original last step) ----
                # Wf streamed in per-g chunks, double-buffered against the
                # MMs; its pools open only after the m-loop pools close.
                with tc.tile_pool(name="p1wf", bufs=1) as p1wf, \
                     tc.tile_pool(name="p1f", bufs=1) as p1f:
                    bif_sb = p1f.tile([BLOC, G3], BF16, tag="bif")
                    nc.gpsimd.dma_start(out=bif_sb[:], in_=bias_f[0:BLOC, :])
                    for g in range(NG):
                        wf_sb = p1wf.tile([128, KT_E, GC], BF16, tag="wf", bufs=2)
                        wf_src = Wf[:, GC * g:GC * (g + 1)].rearrange("(k p) c -> p k c", p=128)
                        qt = KT_E // 4
                        for h4 in range(4):
                            q = nc.scalar if h4 % 2 == 0 else nc.sync
                            q.dma_start(out=wf_sb[:, h4 * qt:(h4 + 1) * qt, :],
                                        in_=wf_src[:, h4 * qt:(h4 + 1) * qt, :])
                        psf = p1_ps.tile([128, GC], F32, space="PSUM", tag="ps")
                        for k in range(KT_E):
                            for c in range(3):
                                nc.tensor.matmul(
                                    psf[0:BLOC, 512 * c:512 * (c + 1)],
                                    lhsT=xt0_sb[:, 128 * k:128 * k + BLOC],
                                    rhs=wf_sb[:, k, 512 * c:512 * (c + 1)],
                                    start=(k == 0), stop=(k == KT_E - 1))
                        gf = p1f.tile([BLOC, GC], F32, tag="gf")
                        nc.vector.tensor_add(gf[:], psf[0:BLOC, :],
                                             bif_sb[:, GC * g:GC * (g + 1)])
                        af = p1f.tile([BLOC, HG], F32, tag="af")
                        bf = p1f.tile([BLOC, HG], F32, tag="bff")
                        cf = p1f.tile([BLOC, HG], F32, tag="cf")
                        nc.scalar.activation(af[:], gf[:, 0:HG],
                                             mybir.ActivationFunctionType.Sigmoid)
                        nc.scalar.activation(bf[:], gf[:, HG:2 * HG],
                                             mybir.ActivationFunctionType.Tanh)
                        nc.scalar.activation(cf[:], gf[:, 2 * HG:3 * HG],
                                             mybir.ActivationFunctionType.Sigmoid)
                        nc.vector.tensor_mul(af[:], af[:], bf[:])
                        nc.scalar.activation(af[:], af[:],
                                             mybir.ActivationFunctionType.Tanh)
                        nc.vector.tensor_mul(af[:], cf[:], af[:])
                        nc.sync.dma_start(out=out[:, HG * g:HG * (g + 1)], in_=af[:])

            # remaining Wr k-tiles, split across both HW DMA queues, in the
            # order step 1 will consume them.
            with tc.tile_pool(name="wr2", bufs=1) as wr2p:
                tc.strict_bb_all_engine_barrier()
                wr2_loads = []
                for i, k in enumerate(K_ORDER[NWR1:]):
                    w_k = wr2p.tile([128, G3], BF16, tag=f"wr{k}")
                    wr2_loads.append((nc.sync if i % 2 == 0 else nc.scalar, w_k, k))
                    wr_tiles[k] = w_k
                # ---------------- phase R: recurrence ----------------
                with tc.tile_pool(name="pr", bufs=2) as pr, \
                     tc.tile_pool(name="pr1", bufs=1) as pr1, \
                     tc.tile_pool(name="prh", bufs=8) as prh, \
                     tc.tile_pool(name="pr_ps", bufs=2, space="PSUM") as pr_ps, \
                     tc.tile_pool(name="prt_ps", bufs=2, space="PSUM") as prt_ps:
                    identb = pr1.tile([128, 128], BF16)
                    make_identity(nc, identb[:])

                    a_t = pr1.tile([128, HG], F32)
                    b_t = pr1.tile([128, HG], F32)

                    def load_xg(s):
                        # split across sync + gpsimd DMA queues; steps 1-2
                        # stay off the sync queue (busy with the wr2 tiles).
                        xg_sb = pr.tile([128, GC], BF16, tag="xgs")
                        for j in range(NG):
                            q = nc.gpsimd if (s in (1, 2) or j >= 2) else nc.sync
                            q.dma_start(
                                out=xg_sb[BLOC * j:BLOC * (j + 1), :],
                                in_=xgd[BLOC * s:BLOC * (s + 1), GC * j:GC * (j + 1)])
                        return xg_sb

                    def issue_folds(xg_sb):
                        """The three xg fold rounds (start=True) for a step:
                        shifted-identity stationary selects xg rows
                        32j..32j+32 for psum region j — exact, and h-
                        independent so the PE chews them during the previous
                        step's act-ladder dependency gap."""
                        ps_i = pr_ps.tile([128, 512], F32, space="PSUM", tag="gps0")
                        ps_g = pr_ps.tile([128, 512], F32, space="PSUM", tag="gps1")
                        ps_o = pr_ps.tile([128, 512], F32, space="PSUM", tag="gps2")
                        for c, ps_c in ((0, ps_i), (1, ps_g), (2, ps_o)):
                            for j in range(NG):
                                nc.tensor.matmul(
                                    ps_c[BLOC * j:BLOC * (j + 1), :],
                                    lhsT=identb[:, BLOC * j:BLOC * (j + 1)],
                                    rhs=xg_sb[:, 512 * c:512 * (c + 1)],
                                    start=True, stop=False,
                                    tile_position=(0, BLOC * j),
                                    skip_group_check=True)
                        return ps_i, ps_g, ps_o

                    def act_part1(gi_ap, gg_ap):
                        """v = tanh(sig(i)*tanh(g)) -> a_t. Runs on ACT/DVE while
                        the o-bank matmuls stream on the PE."""
                        nc.scalar.activation(a_t[:], gi_ap,
                                             mybir.ActivationFunctionType.Sigmoid)
                        nc.scalar.activation(b_t[:], gg_ap,
                                             mybir.ActivationFunctionType.Tanh)
                        nc.vector.tensor_mul(a_t[:], a_t[:], b_t[:])      # u = sig(i)*tanh(g)
                        nc.scalar.activation(a_t[:], a_t[:],
                                             mybir.ActivationFunctionType.Tanh)  # v

                    def act_part2(go_ap, store_out=False, pe_only=False):
                        if store_out:
                            nc.scalar.activation(b_t[:], go_ap,
                                                 mybir.ActivationFunctionType.Sigmoid)
                            h_t = pr.tile([128, HG], F32, tag="hfin", bufs=1)
                            nc.vector.tensor_mul(h_t[:], b_t[:], a_t[:])
                            for j in range(NG):
                                nc.sync.dma_start(
                                    out=out[:, HID + HG * j:HID + HG * (j + 1)],
                                    in_=h_t[BLOC * j:BLOC * (j + 1), :])
                            return None
                        # chunk-pipelined: per-128-col sigmoid -> mul -> PE
                        # transpose -> copy, so the first transpose starts
                        # ~0.35us earlier than a full-width sigmoid allows
                        # chunk 0 transposes on the PE (shortest latency to
                        # unblock the next sweep); chunks 1-3 on the DMA xbar
                        # whose latency hides behind the chunk-0 matmul groups.
                        # The xbar issues go AFTER the whole sig/mul ladder in
                        # program order: a DMA's semaphore wait blocks the
                        # issuing engine's queue, and it must not hold up the
                        # later sigmoids.
                        hTs, deferred = [], []
                        for q in range(NG):
                            sl = slice(128 * q, 128 * (q + 1))
                            b_q = pr.tile([128, 128], BF16, tag="bq", bufs=2)
                            nc.scalar.activation(b_q[:], go_ap[:, sl],
                                                 mybir.ActivationFunctionType.Sigmoid)
                            h_q = pr.tile([128, 128], BF16, tag="h", bufs=3)
                            nc.vector.tensor_mul(h_q[:], b_q[:], a_t[:, sl])
                            hT_q = prh.tile([128, 128], BF16, tag="hT")
                            if q == 0 or pe_only:
                                t_ps = prt_ps.tile([128, 128], BF16, space="PSUM", tag="tps")
                                nc.tensor.transpose(out=t_ps[:], in_=h_q[:],
                                                    identity=identb[:])
                                nc.vector.tensor_copy(hT_q[:], t_ps[:])
                            else:
                                deferred.append((q, h_q, hT_q))
                            hTs.append(hT_q)
                        for q, h_q, hT_q in deferred:
                            nc.sync.dma_start_transpose(out=hT_q[:], in_=h_q[:])
                        return hTs

                    # step 0: h=0 -> gates are just xg. Issued BEFORE the
                    # wr2 weight DMAs so its ladder isn't queued behind their
                    # semaphore waits on the hw DMA queues.
                    xg0 = load_xg(0)
                    act_part1(xg0[:, 0:HG], xg0[:, HG:2 * HG])
                    hT = act_part2(xg0[:, 2 * HG:3 * HG], pe_only=True)
                    # step 1 folds go into the PE queue right behind step 0's
                    # transposes
                    xg_nxt = load_xg(1)
                    folds = issue_folds(xg_nxt)
                    for q, w_k, k in wr2_loads:
                        q.dma_start(out=w_k[:], in_=Wr[128 * k:128 * (k + 1), :])

                    for s in range(1, n_steps):
                        ps_i, ps_g, ps_o = folds
                        # sweep 1: i and g banks, k-outer so each hT stationary
                        # load serves 8 matmuls; o bank in a second sweep so the
                        # i/g activation chain overlaps the o matmul stream.
                        for ki, k in enumerate(K_ORDER):
                            lhs = hT[k % NG][:, BLOC * (k // NG):BLOC * (k // NG) + BLOC]
                            for c, ps_c in ((0, ps_i), (1, ps_g)):
                                for j in range(NG):
                                    nc.tensor.matmul(
                                        ps_c[BLOC * j:BLOC * (j + 1), :],
                                        lhsT=lhs,
                                        rhs=wr_tiles[k][:, GC * j + 512 * c:GC * j + 512 * (c + 1)],
                                        start=False, stop=(ki == KT_H - 1),
                                        tile_position=(0, BLOC * j),
                                        skip_group_check=True)
                        # sweep 2: o bank (xg contribution already folded in
                        # as the start=True round).
                        for ki, k in enumerate(K_ORDER):
                            lhs = hT[k % NG][:, BLOC * (k // NG):BLOC * (k // NG) + BLOC]
                            for j in range(NG):
                                nc.tensor.matmul(
                                    ps_o[BLOC * j:BLOC * (j + 1), :],
                                    lhsT=lhs,
                                    rhs=wr_tiles[k][:, GC * j + 1024:GC * j + 1536],
                                    start=False, stop=(ki == KT_H - 1),
                                    tile_position=(0, BLOC * j),
                                    skip_group_check=True)
                        # next step's xg load + folds enter the PE queue HERE —
                        # before this step's transposes — so they fill the
                        # act-ladder dependency gap (PE queue is strict FIFO).
                        if s + 1 < n_steps:
                            xg_nxt = load_xg(s + 1)
                            folds = issue_folds(xg_nxt)
                        act_part1(ps_i[:], ps_g[:])
                        hT = act_part2(ps_o[:], store_out=(s == n_steps - 1),
                                       pe_only=(s <= 2))
    nc.compile()
    return nc


_BUILT = {}


def _get_built(n_steps=None):
    key = n_steps or N_STEPS
    if key not in _BUILT:
        _BUILT[key] = build(key)
    return _BUILT[key]


def _perm():
    """Row permutation taking PyTorch (i|f|g|o)*2048 rows to 4 groups of
    (i|g|o)*512."""
    p = []
    for j in range(NG):
        for base in (0, 2 * HID, 3 * HID):  # i, g, o blocks
            p.extend(range(base + HG * j, base + HG * (j + 1)))
    return np.array(p)


def prep_inputs(inputs, embed_table, W_ih_f, W_hh_f, b_ih_f, b_hh_f,
                W_ih_b, W_hh_b, b_ih_b, b_hh_b):
    perm = _perm()
    idx = np.asarray(inputs)
    idx = np.where(idx > VOCAB, 0, idx).astype(np.int64)
    idx = np.clip(idx, 0, VOCAB - 1).astype(np.int32)

    Wi_p = np.ascontiguousarray(
        np.asarray(W_ih_b)[perm].T.astype(ml_dtypes.bfloat16))
    Wf_p = np.ascontiguousarray(
        np.asarray(W_ih_f)[perm].T.astype(ml_dtypes.bfloat16))
    Wr_p = np.ascontiguousarray(
        np.asarray(W_hh_b)[perm].T.astype(ml_dtypes.bfloat16))
    bb = (np.asarray(b_ih_b) + np.asarray(b_hh_b))[perm].astype(np.float32)
    bf = (np.asarray(b_ih_f) + np.asarray(b_hh_f))[perm].astype(np.float32)
    bias_b_t = np.ascontiguousarray(
        np.broadcast_to(bb, (128, G3)).astype(ml_dtypes.bfloat16))
    bias_f_t = np.ascontiguousarray(
        np.broadcast_to(bf, (128, G3)).astype(ml_dtypes.bfloat16))
    table = np.ascontiguousarray(
        np.asarray(embed_table).astype(ml_dtypes.bfloat16))

    in_maps = []
    for c in range(NCORES):
        sl = idx[BLOC * c:BLOC * (c + 1)]          # [32, 128]
        tok = np.ascontiguousarray(sl[:, ::-1].T.reshape(NTOK, 1))  # t-major rev
        in_maps.append({
            "tok": tok, "table": table, "Wi": Wi_p, "Wf": Wf_p, "Wr": Wr_p,
            "bias_b": bias_b_t, "bias_f": bias_f_t,
        })
    return in_maps


def kernel(**inputs) -> np.ndarray:
    from concourse.bass_utils import run_bass_kernel_spmd
    nc = _get_built()
    in_maps = prep_inputs(**inputs)
    res = run_bass_kernel_spmd(nc, in_maps, core_ids=list(range(NCORES)))
    return np.concatenate([res.results[c]["out"] for c in range(NCORES)], axis=0)


# revision 11
# speedup vs baseline: 1.3017x; 1.2362x over previous
"""Bidirectional-LSTM (degenerate variant) Trainium2 kernel.

Reference semantics (see harness): for the forward direction only the last
timestep matters (h/c never update), and the backward direction is an
h-only recurrence (c stays zero), so only the i/g/o gates are ever used:

    h_fwd = sig(o) * tanh(sig(i) * tanh(g)),  gates = x_last @ W_ih_f.T + b_f
    h_bwd: scan t = S-1..0 with
        gates = x_t @ W_ih_b.T + b_b + h @ W_hh_b.T   (f-gate unused)
        h     = sig(o) * tanh(sig(i) * tanh(g))
    out = [h_fwd | h_bwd]  -> [256, 4096]

Distribution: pure data-parallel over batch (32 per core, 8 cores), weights
replicated. Per core:
  phase 1: m-outer fused pipeline — embedding gather (indirect DMA, table
           pre-cast to bf16 on host) -> PE-transpose -> input projection
           xg = X @ Wi + b (bf16, Wi SBUF-resident) -> xg to DRAM; fwd cell
           at the end with Wf streamed per-g. Wr recurrence weights are
           prefetched k-tile-granular: 6 tiles trickle in on the scalar DMA
           queue during the m-loop, the rest right after the fwd cell, in
           step-1 consumption order, so phase R starts with almost no
           weight-load bubble.
  phase R: 128-step recurrence. gates = Wr.T @ h via 4 col-tiled concurrent
           M=32 matmuls (bf16), two chunk-major k-order sweeps (i,g then o).
           All three xg contributions enter the PE as start=True
           shifted-identity fold rounds issued one step AHEAD (before the
           previous step's transposes in the PE FIFO), so they execute in
           the act-ladder dependency gap; xg tiles double-buffer and load
           split across the sync+gpsimd DMA queues a full step early.
           The i/g activation chain hides under the o matmul stream; h is
           re-transposed per 128-col chunk so the next sweep starts as the
           first transposed chunk lands.

Gate columns are host-permuted into 4 groups of (i|g|o) x 512 hid dims so
each PSUM column-group j directly yields h[:, 512j:512j+512].
"""

import numpy as np
import ml_dtypes

import concourse.bass as bass
import concourse.bacc as bacc
import concourse.mybir as mybir
import concourse.tile as tile
from concourse.masks import make_identity

VOCAB, EMB, HID = 50000, 1024, 2048
BATCH, SEQ = 256, 128
NCORES = 8
BLOC = BATCH // NCORES            # 32 batch rows per core
NTOK = BLOC * SEQ                 # 4096 tokens per core
NG = 4                            # PSUM column groups
GC = 3 * HID // NG                # 1536 gate cols per group (i|g|o x 512)
HG = HID // NG                    # 512 hid dims per group
G3 = 3 * HID                      # 6144 total igo gate cols
MT = NTOK // 128                  # 32 token m-tiles
KT_E = EMB // 128                 # 8 k-tiles for input projection
KT_H = HID // 128                 # 16 k-tiles for recurrence
NWR1 = 6                          # Wr k-tiles prefetched during m-loop

F32 = mybir.dt.float32
BF16 = mybir.dt.bfloat16
I32 = mybir.dt.int32

N_STEPS = SEQ  # overridable for mini builds

# chunk-major k order: the 4 k-tiles living in hT chunk 0 run first, so a
# sweep starts at full rate as soon as the previous step's first transposed
# chunk lands. Also the Wr k-tile DMA issue order.
K_ORDER = [q + NG * r for q in range(NG) for r in range(NG)]


def build(n_steps=None):
    n_steps = n_steps or N_STEPS
    nc = bacc.Bacc("TRN2", target_bir_lowering=False, debug=False,
                   num_devices=NCORES)

    tok = nc.dram_tensor("tok", [NTOK, 1], I32, kind="ExternalInput")
    table = nc.dram_tensor("table", [VOCAB, EMB], BF16, kind="ExternalInput")
    Wi = nc.dram_tensor("Wi", [EMB, G3], BF16, kind="ExternalInput")
    Wf = nc.dram_tensor("Wf", [EMB, G3], BF16, kind="ExternalInput")
    Wr = nc.dram_tensor("Wr", [HID, G3], BF16, kind="ExternalInput")
    bias_b = nc.dram_tensor("bias_b", [128, G3], BF16, kind="ExternalInput")
    bias_f = nc.dram_tensor("bias_f", [128, G3], BF16, kind="ExternalInput")
    out = nc.dram_tensor("out", [BLOC, 2 * HID], F32, kind="ExternalOutput")

    xgd = nc.dram_tensor("xgd", [NTOK, G3], BF16)         # internal

    wr_tiles = {}

    with tile.TileContext(nc) as tc:
        with tc.tile_pool(name="wr1", bufs=1) as wr1p:
            # ------- phase 1: gather + transpose + input projection -------
            with tc.tile_pool(name="p1s", bufs=1) as p1s, \
                 tc.tile_pool(name="p1_ps", bufs=2, space="PSUM") as p1_ps:
                with tc.tile_pool(name="p1w", bufs=1) as p1w, \
                     tc.tile_pool(name="p1", bufs=2) as p1, \
                     tc.tile_pool(name="p1x", bufs=2) as p1x, \
                     tc.tile_pool(name="p1t_ps", bufs=2, space="PSUM") as p1t_ps:
                    ident = p1s.tile([128, 128], BF16)
                    make_identity(nc, ident[:])
                    # first two gathers go out before the weight loads; Wi is
                    # split across the sync+scalar DMA queues so the m-loop
                    # can start ~35us in.
                    pre_idx, pre_x = [], []
                    for m in range(2):
                        idx_sb = p1.tile([128, 1], I32, tag="idx")
                        nc.sync.dma_start(out=idx_sb[:], in_=tok[m * 128:(m + 1) * 128, :])
                        x_sb = p1x.tile([128, EMB], BF16, tag="x")
                        nc.gpsimd.indirect_dma_start(
                            out=x_sb[:], out_offset=None, in_=table[:, :],
                            in_offset=bass.IndirectOffsetOnAxis(ap=idx_sb[:, :1], axis=0))
                        pre_idx.append(idx_sb)
                        pre_x.append(x_sb)
                    wi_sb = []
                    for g in range(NG):
                        w_g = p1w.tile([128, KT_E, GC], BF16, tag=f"wi{g}")
                        q = nc.sync if g in (0, 2) else nc.scalar
                        q.dma_start(
                            out=w_g[:],
                            in_=Wi[:, GC * g:GC * (g + 1)].rearrange("(k p) c -> p k c", p=128))
                        wi_sb.append(w_g)
                    bia_sb = p1s.tile([128, G3], BF16, tag="bia")
                    nc.scalar.dma_start(out=bia_sb[:], in_=bias_b[:, :])
                    # Wr k-tile prefetch: 6 tiles trickle in on the scalar
                    # queue (behind Wi g1/g3+bias) while the m-loop runs.
                    for k in K_ORDER[:NWR1]:
                        w_k = wr1p.tile([128, G3], BF16, tag=f"wr{k}")
                        nc.scalar.dma_start(out=w_k[:], in_=Wr[128 * k:128 * (k + 1), :])
                        wr_tiles[k] = w_k
                    xt0_sb = p1s.tile([128, EMB], BF16)

                    for m in range(MT):
                        if m < 2:
                            x_sb = pre_x[m]
                        else:
                            # idx+gather both on the gpsimd queue (FIFO keeps
                            # them ordered) so the sync queue stays free for
                            # xgd writes.
                            idx_sb = p1.tile([128, 1], I32, tag="idx")
                            nc.gpsimd.dma_start(out=idx_sb[:], in_=tok[m * 128:(m + 1) * 128, :])
                            x_sb = p1x.tile([128, EMB], BF16, tag="x")
                            nc.gpsimd.indirect_dma_start(
                                out=x_sb[:], out_offset=None, in_=table[:, :],
                                in_offset=bass.IndirectOffsetOnAxis(ap=idx_sb[:, :1], axis=0))
                        xt_sb = p1x.tile([128, EMB], BF16, tag="xt")
                        for q in range(KT_E):
                            t_ps = p1t_ps.tile([128, 128], BF16, space="PSUM", tag="tps")
                            nc.tensor.transpose(out=t_ps[:], in_=x_sb[:, 128 * q:128 * (q + 1)],
                                                identity=ident[:])
                            nc.vector.tensor_copy(xt_sb[:, 128 * q:128 * (q + 1)], t_ps[:])
                        if m == 0:
                            nc.vector.tensor_copy(xt0_sb[:], xt_sb[:])
                        for g in range(NG):
                            ps = p1_ps.tile([128, GC], F32, space="PSUM", tag="ps")
                            for k in range(KT_E):
                                for c in range(3):
                                    nc.tensor.matmul(
                                        ps[:, 512 * c:512 * (c + 1)],
                                        lhsT=xt_sb[:, 128 * k:128 * (k + 1)],
                                        rhs=wi_sb[g][:, k, 512 * c:512 * (c + 1)],
                                        start=(k == 0), stop=(k == KT_E - 1))
                            xg_sb = p1.tile([128, GC], BF16, tag="xg")
                            nc.vector.tensor_add(xg_sb[:], ps[:],
                                                 bia_sb[:, GC * g:GC * (g + 1)])
                            nc.sync.dma_start(
                                out=xgd[m * 128:(m + 1) * 128, GC * g:GC * (g + 1)],
                                in_=xg_sb[:])

                # ---- forward cell (tokens 0..32 = original last step) ----
                # Wf streamed in per-g chunks, double-buffered against the
                # MMs; its pools open only after the m-loop pools close.
                with tc.tile_pool(name="p1wf", bufs=1) as p1wf, \
                     tc.tile_pool(name="p1f", bufs=1) as p1f:
                    bif_sb = p1f.tile([BLOC, G3], BF16, tag="bif")
                    nc.gpsimd.dma_start(out=bif_sb[:], in_=bias_f[0:BLOC, :])
                    for g in range(NG):
                        wf_sb = p1wf.tile([128, KT_E, GC], BF16, tag="wf", bufs=2)
                        wf_src = Wf[:, GC * g:GC * (g + 1)].rearrange("(k p) c -> p k c", p=128)
                        qt = KT_E // 4
                        for h4 in range(4):
                            q = nc.scalar if h4 % 2 == 0 else nc.sync
                            q.dma_start(out=wf_sb[:, h4 * qt:(h4 + 1) * qt, :],
                                        in_=wf_src[:, h4 * qt:(h4 + 1) * qt, :])
                        psf = p1_ps.tile([128, GC], F32, space="PSUM", tag="ps")
                        for k in range(KT_E):
                            for c in range(3):
                                nc.tensor.matmul(
                                    psf[0:BLOC, 512 * c:512 * (c + 1)],
                                    lhsT=xt0_sb[:, 128 * k:128 * k + BLOC],
                                    rhs=wf_sb[:, k, 512 * c:512 * (c + 1)],
                                    start=(k == 0), stop=(k == KT_E - 1))
                        gf = p1f.tile([BLOC, GC], F32, tag="gf")
                        nc.vector.tensor_add(gf[:], psf[0:BLOC, :],
                                             bif_sb[:, GC * g:GC * (g + 1)])
                        af = p1f.tile([BLOC, HG], F32, tag="af")
                        bf = p1f.tile([BLOC, HG], F32, tag="bff")
                        cf = p1f.tile([BLOC, HG], F32, tag="cf")
                        nc.scalar.activation(af[:], gf[:, 0:HG],
                                             mybir.ActivationFunctionType.Sigmoid)
                        nc.scalar.activation(bf[:], gf[:, HG:2 * HG],
                                             mybir.ActivationFunctionType.Tanh)
                        nc.scalar.activation(cf[:], gf[:, 2 * HG:3 * HG],
                                             mybir.ActivationFunctionType.Sigmoid)
                        nc.vector.tensor_mul(af[:], af[:], bf[:])
                        nc.scalar.activation(af[:], af[:],
                                             mybir.ActivationFunctionType.Tanh)
                        nc.vector.tensor_mul(af[:], cf[:], af[:])
                        nc.sync.dma_start(out=out[:, HG * g:HG * (g + 1)], in_=af[:])

            # remaining Wr k-tiles, split across both HW DMA queues, in the
            # order step 1 will consume them.
            with tc.tile_pool(name="wr2", bufs=1) as wr2p:
                tc.strict_bb_all_engine_barrier()
                wr2_loads = []
                for i, k in enumerate(K_ORDER[NWR1:]):
                    w_k = wr2p.tile([128, G3], BF16, tag=f"wr{k}")
                    wr2_loads.append((nc.sync if i % 2 == 0 else nc.scalar, w_k, k))
                    wr_tiles[k] = w_k
                # ---------------- phase R: recurrence ----------------
                with tc.tile_pool(name="pr", bufs=2) as pr, \
                     tc.tile_pool(name="pr1", bufs=1) as pr1, \
                     tc.tile_pool(name="prh", bufs=8) as prh, \
                     tc.tile_pool(name="pr_ps", bufs=2, space="PSUM") as pr_ps, \
                     tc.tile_pool(name="prt_ps", bufs=2, space="PSUM") as prt_ps:
                    identb = pr1.tile([128, 128], BF16)
                    make_identity(nc, identb[:])

                    a_t = pr1.tile([128, HG], F32)
                    b_t = pr1.tile([128, HG], F32)

                    def load_xg(s):
                        # split across sync + gpsimd DMA queues; steps 1-2
                        # stay off the sync queue (busy with the wr2 tiles).
                        xg_sb = pr.tile([128, GC], BF16, tag="xgs")
                        for j in range(NG):
                            q = nc.gpsimd if (s in (1, 2) or j >= 2) else nc.sync
                            q.dma_start(
                                out=xg_sb[BLOC * j:BLOC * (j + 1), :],
                                in_=xgd[BLOC * s:BLOC * (s + 1), GC * j:GC * (j + 1)])
                        return xg_sb

                    def issue_folds(xg_sb):
                        """The three xg fold rounds (start=True) for a step:
                        shifted-identity stationary selects xg rows
                        32j..32j+32 for psum region j — exact, and h-
                        independent so the PE chews them during the previous
                        step's act-ladder dependency gap."""
                        ps_i = pr_ps.tile([128, 512], F32, space="PSUM", tag="gps0")
                        ps_g = pr_ps.tile([128, 512], F32, space="PSUM", tag="gps1")
                        ps_o = pr_ps.tile([128, 512], F32, space="PSUM", tag="gps2")
                        for c, ps_c in ((0, ps_i), (1, ps_g), (2, ps_o)):
                            for j in range(NG):
                                nc.tensor.matmul(
                                    ps_c[BLOC * j:BLOC * (j + 1), :],
                                    lhsT=identb[:, BLOC * j:BLOC * (j + 1)],
                                    rhs=xg_sb[:, 512 * c:512 * (c + 1)],
                                    start=True, stop=False,
                                    tile_position=(0, BLOC * j),
                                    skip_group_check=True)
                        return ps_i, ps_g, ps_o

                    def act_part1(gi_ap, gg_ap):
                        """v = tanh(sig(i)*tanh(g)) -> a_t. Runs on ACT/DVE while
                        the o-bank matmuls stream on the PE."""
                        nc.scalar.activation(a_t[:], gi_ap,
                                             mybir.ActivationFunctionType.Sigmoid)
                        nc.scalar.activation(b_t[:], gg_ap,
                                             mybir.ActivationFunctionType.Tanh)
                        nc.vector.tensor_mul(a_t[:], a_t[:], b_t[:])      # u = sig(i)*tanh(g)
                        nc.scalar.activation(a_t[:], a_t[:],
                                             mybir.ActivationFunctionType.Tanh)  # v

                    def act_part2(go_ap, store_out=False):
                        if store_out:
                            nc.scalar.activation(b_t[:], go_ap,
                                                 mybir.ActivationFunctionType.Sigmoid)
                            h_t = pr.tile([128, HG], F32, tag="hfin", bufs=1)
                            nc.vector.tensor_mul(h_t[:], b_t[:], a_t[:])
                            for j in range(NG):
                                nc.sync.dma_start(
                                    out=out[:, HID + HG * j:HID + HG * (j + 1)],
                                    in_=h_t[BLOC * j:BLOC * (j + 1), :])
                            return None
                        # chunk-pipelined: per-128-col sigmoid -> mul -> PE
                        # transpose -> copy, so the first transpose starts
                        # ~0.35us earlier than a full-width sigmoid allows
                        # all four transposes on the PE: xbar-DMA variants
                        # kept losing to queue-blocking semantics.
                        hTs = []
                        for q in range(NG):
                            sl = slice(128 * q, 128 * (q + 1))
                            b_q = pr.tile([128, 128], BF16, tag="bq", bufs=2)
                            nc.scalar.activation(b_q[:], go_ap[:, sl],
                                                 mybir.ActivationFunctionType.Sigmoid)
                            h_q = pr.tile([128, 128], BF16, tag="h", bufs=3)
                            nc.vector.tensor_mul(h_q[:], b_q[:], a_t[:, sl])
                            t_ps = prt_ps.tile([128, 128], BF16, space="PSUM", tag="tps")
                            nc.tensor.transpose(out=t_ps[:], in_=h_q[:],
                                                identity=identb[:])
                            hT_q = prh.tile([128, 128], BF16, tag="hT")
                            nc.vector.tensor_copy(hT_q[:], t_ps[:])
                            hTs.append(hT_q)
                        return hTs

                    # step 0: h=0 -> gates are just xg. Issued BEFORE the
                    # wr2 weight DMAs so its ladder isn't queued behind their
                    # semaphore waits on the hw DMA queues.
                    xg0 = load_xg(0)
                    act_part1(xg0[:, 0:HG], xg0[:, HG:2 * HG])
                    hT = act_part2(xg0[:, 2 * HG:3 * HG])
                    # step 1 folds go into the PE queue right behind step 0's
                    # transposes
                    xg_nxt = load_xg(1)
                    folds = issue_folds(xg_nxt)
                    for q, w_k, k in wr2_loads:
                        q.dma_start(out=w_k[:], in_=Wr[128 * k:128 * (k + 1), :])

                    for s in range(1, n_steps):
                        ps_i, ps_g, ps_o = folds
                        # sweep 1: i and g banks, k-outer so each hT stationary
                        # load serves 8 matmuls; o bank in a second sweep so the
                        # i/g activation chain overlaps the o matmul stream.
                        for ki, k in enumerate(K_ORDER):
                            lhs = hT[k % NG][:, BLOC * (k // NG):BLOC * (k // NG) + BLOC]
                            for c, ps_c in ((0, ps_i), (1, ps_g)):
                                for j in range(NG):
                                    nc.tensor.matmul(
                                        ps_c[BLOC * j:BLOC * (j + 1), :],
                                        lhsT=lhs,
                                        rhs=wr_tiles[k][:, GC * j + 512 * c:GC * j + 512 * (c + 1)],
                                        start=False, stop=(ki == KT_H - 1),
                                        tile_position=(0, BLOC * j),
                                        skip_group_check=True)
                        # sweep 2: o bank (xg contribution already folded in
                        # as the start=True round).
                        for ki, k in enumerate(K_ORDER):
                            lhs = hT[k % NG][:, BLOC * (k // NG):BLOC * (k // NG) + BLOC]
                            for j in range(NG):
                                nc.tensor.matmul(
                                    ps_o[BLOC * j:BLOC * (j + 1), :],
                                    lhsT=lhs,
                                    rhs=wr_tiles[k][:, GC * j + 1024:GC * j + 1536],
                                    start=False, stop=(ki == KT_H - 1),
                                    tile_position=(0, BLOC * j),
                                    skip_group_check=True)
                        # next step's xg load + folds enter the PE queue HERE —
                        # before this step's transposes — so they fill the
                        # act-ladder dependency gap (PE queue is strict FIFO).
                        if s + 1 < n_steps:
                            xg_nxt = load_xg(s + 1)
                            folds = issue_folds(xg_nxt)
                        act_part1(ps_i[:], ps_g[:])
                        hT = act_part2(ps_o[:], store_out=(s == n_steps - 1))
    nc.compile()
    return nc


_BUILT = {}


def _get_built(n_steps=None):
    key = n_steps or N_STEPS
    if key not in _BUILT:
        _BUILT[key] = build(key)
    return _BUILT[key]


def _perm():
    """Row permutation taking PyTorch (i|f|g|o)*2048 rows to 4 groups of
    (i|g|o)*512."""
    p = []
    for j in range(NG):
        for base in (0, 2 * HID, 3 * HID):  # i, g, o blocks
            p.extend(range(base + HG * j, base + HG * (j + 1)))
    return np.array(p)


def prep_inputs(inputs, embed_table, W_ih_f, W_hh_f, b_ih_f, b_hh_f,
                W_ih_b, W_hh_b, b_ih_b, b_hh_b):
    perm = _perm()
    idx = np.asarray(inputs)
    idx = np.where(idx > VOCAB, 0, idx).astype(np.int64)
    idx = np.clip(idx, 0, VOCAB - 1).astype(np.int32)

    Wi_p = np.ascontiguousarray(
        np.asarray(W_ih_b)[perm].T.astype(ml_dtypes.bfloat16))
    Wf_p = np.ascontiguousarray(
        np.asarray(W_ih_f)[perm].T.astype(ml_dtypes.bfloat16))
    Wr_p = np.ascontiguousarray(
        np.asarray(W_hh_b)[perm].T.astype(ml_dtypes.bfloat16))
    bb = (np.asarray(b_ih_b) + np.asarray(b_hh_b))[perm].astype(np.float32)
    bf = (np.asarray(b_ih_f) + np.asarray(b_hh_f))[perm].astype(np.float32)
    bias_b_t = np.ascontiguousarray(
        np.broadcast_to(bb, (128, G3)).astype(ml_dtypes.bfloat16))
    bias_f_t = np.ascontiguousarray(
        np.broadcast_to(bf, (128, G3)).astype(ml_dtypes.bfloat16))
    table = np.ascontiguousarray(
        np.asarray(embed_table).astype(ml_dtypes.bfloat16))

    in_maps = []
    for c in range(NCORES):
        sl = idx[BLOC * c:BLOC * (c + 1)]          # [32, 128]
        tok = np.ascontiguousarray(sl[:, ::-1].T.reshape(NTOK, 1))  # t-major rev
        in_maps.append({
            "tok": tok, "table": table, "Wi": Wi_p, "Wf": Wf_p, "Wr": Wr_p,
            "bias_b": bias_b_t, "bias_f": bias_f_t,
        })
    return in_maps


def kernel(**inputs) -> np.ndarray:
    from concourse.bass_utils import run_bass_kernel_spmd
    nc = _get_built()
    in_maps = prep_inputs(**inputs)
    res = run_bass_kernel_spmd(nc, in_maps, core_ids=list(range(NCORES)))
    return np.concatenate([res.results[c]["out"] for c in range(NCORES)], axis=0)
